# revision 20
# baseline (speedup 1.0000x reference)
"""ClusterNorm1d TRN2 kernel (pair-batched, low-precision, phase-pipelined).

Math (per cluster k): mu = mean_b x[b,:,k]; cov = centered second moment;
L = chol(cov + eps I); Z = L^-1 (x - mu).

Per core: 32 clusters processed as 16 PAIRS (two 64-dim clusters batched
block-diagonally into 128-wide PE ops).

  - stats: per pair, 32 accumulating bf16 matmuls G += U_j^T [U_j | 1]
    ([128,129] out in fp32 PSUM).  Diag 64x64 blocks of G[:, :128] are the
    two clusters' Grams; col 128 is the per-column sum s (mean * B).
  - cov -> W = L^-1 via Newton on the Cholesky manifold, all in fp16
    (1 cyc/row on PE vs 4 for fp32; iteration is self-correcting):
      P = W A W^T;  I + C^T = 1.5 I - CM o P;  W <- (I+C) W
    keeping both W and W^T via two matmuls per step (no transposes).
  - solve: Z = W x - (W mu) 1^T as bf16 matmuls, mean applied as
    per-partition bias during the PSUM -> SBUF copy; output stored bf16.
Inputs shipped bf16 twice (b-major for stats, d-major for solve), output
bf16: ~51 MB HBM traffic per core vs 85 MB for the f32 layout.
"""
import sys
sys.path.insert(0, "/opt/trn_rl_repo")

import numpy as np
import ml_dtypes

import concourse.bass as bass
from concourse import bacc
import concourse.mybir as mybir
import concourse.tile as tile
from concourse.bass_utils import run_bass_kernel_spmd

B, D, K, NCORES = 4096, 64, 256, 8
KL = K // NCORES          # clusters per core (32)
NP = KL // 2              # cluster pairs per core (16)
NCH = B // 128            # stats chunks (32)
NB = B // 512             # solve chunks per pair (8)
EPS = 1e-4
NIT = 2                   # newton steps incl analytic it0
AF = mybir.ActivationFunctionType
F32 = mybir.dt.float32
F16 = mybir.dt.float16
BF16 = mybir.dt.bfloat16
F8 = mybir.dt.float8e4

_cache = {}


def _build_nc():
    nc = bacc.Bacc("TRN2", target_bir_lowering=False, debug=False,
                   num_devices=NCORES)
    d_xb = nc.dram_tensor("xb", [NP, 128, NCH * 130], F8,
                          kind="ExternalInput")
    d_xs = nc.dram_tensor("xs", [NP, 128, B], BF16, kind="ExternalInput")
    d_cs = nc.dram_tensor("cs", [128, 5 * 128], F32, kind="ExternalInput")
    d_id = nc.dram_tensor("id16", [128, 128], F16, kind="ExternalInput")
    d_out = nc.dram_tensor("out", [NP, 128, B], BF16, kind="ExternalOutput")

    inv_b = 1.0 / B
    a_cov = 1.0 / (B - 1)
    sq_bcov = float(np.sqrt(1.0 / (B * (B - 1.0))))

    with tile.TileContext(nc) as tc:
        with tc.tile_pool(name="consts", bufs=1) as consts, \
             tc.tile_pool(name="ubp", bufs=3) as ubp, \
             tc.tile_pool(name="slabp", bufs=12) as slabp, \
             tc.tile_pool(name="zp", bufs=3) as zp, \
             tc.tile_pool(name="smf", bufs=3) as smf, \
             tc.tile_pool(name="s16", bufs=3) as s16, \
             tc.tile_pool(name="wp", bufs=2 * NP + 4) as wp, \
             tc.tile_pool(name="pp", bufs=NP) as pp, \
             tc.tile_pool(name="ps_big", bufs=2, space="PSUM") as ps_big, \
             tc.tile_pool(name="ps_sm", bufs=4, space="PSUM") as ps_sm, \
             tc.tile_pool(name="ps_row", bufs=1, space="PSUM") as ps_row:

            cs = consts.tile([128, 5 * 128], F32)
            nc.sync.dma_start(out=cs, in_=d_cs.ap())
            cmask = cs[:, 0:128]          # blkdiag(triu(64,1) + 0.5 I)
            blkm = cs[:, 128:256]         # blkdiag(ones 64x64)
            epsi = cs[:, 256:384]         # EPS * I
            c15 = cs[:, 384:512]          # 1.5 * I
            c15e = cs[:, 512:640]         # (1.5 - 0.5*EPS) * I
            id16 = consts.tile([128, 128], F16)
            nc.sync.dma_start(out=id16, in_=d_id.ap())

            P = [dict() for _ in range(NP)]

            # ---------------- Phase A: stats + cov prep ----------------
            def emit_prep(p):
                psg = P[p]["psg"]
                srow = s16.tile([128, 1], F16, tag="srow")
                nc.scalar.activation(out=srow, in_=psg[:, 128:129],
                                     func=AF.Identity, scale=sq_bcov)
                mur = pp.tile([128, 1], BF16, tag="mur")
                nc.scalar.activation(out=mur, in_=psg[:, 128:129],
                                     func=AF.Identity, scale=inv_b)
                pst = ps_row.tile([1, 128], F16, tag="pst")
                nc.tensor.transpose(pst, srow, id16)
                z2 = s16.tile([1, 128], F16, tag="z2")
                nc.scalar.copy(z2, pst)
                pso = ps_sm.tile([128, 128], F32, tag="sm")
                nc.tensor.matmul(pso, z2, z2, start=True, stop=True)
                t1 = smf.tile([128, 128], F32, tag="t1")
                nc.vector.tensor_scalar_mul(t1, psg[:, 0:128], a_cov)
                t3 = smf.tile([128, 128], F32, tag="t3")
                nc.vector.tensor_sub(t3, t1, pso)
                u1 = smf.tile([128, 128], F32, tag="u1")
                nc.vector.tensor_mul(u1, cmask, t3)
                w1t = wp.tile([128, 128], F16, tag="wt")
                nc.vector.tensor_sub(w1t, c15e, u1)
                am = smf.tile([128, 128], F32, tag="am")
                nc.vector.tensor_mul(am, blkm, t3)
                amat = pp.tile([128, 128], F16, tag="amat")
                nc.vector.tensor_add(amat, am, epsi)
                P[p].update(wt=w1t, amat=amat, mur=mur)

            def emit_stats(p):
                ub = ubp.tile([128, NCH * 130], F8, tag="ub")
                nc.scalar.dma_start(out=ub, in_=d_xb.ap()[p])
                psg = ps_big.tile([128, 512], F32, tag="big")
                for j in range(NCH):
                    nc.tensor.matmul(psg[:, 0:129],
                                     ub[:, 130 * j:130 * j + 128],
                                     ub[:, 130 * j:130 * j + 129],
                                     start=(j == 0), stop=(j == NCH - 1))
                P[p]["psg"] = psg

            # ---- Phase B (Newton, step-interleaved within a group) ----
            def emit_newton(group):
                for p in group:
                    psw1 = ps_sm.tile([128, 128], F32, tag="sm")
                    nc.tensor.matmul(psw1, P[p]["wt"], id16, start=True,
                                     stop=True)
                    w = wp.tile([128, 128], F16, tag="w")
                    nc.scalar.copy(w, psw1)
                    P[p]["w"] = w
                for it in range(1, NIT):
                    last = (it == NIT - 1)
                    for p in group:
                        psh = ps_sm.tile([128, 128], F32, tag="sm")
                        nc.tensor.matmul(psh, P[p]["amat"], P[p]["wt"],
                                         start=True, stop=True)
                        h = s16.tile([128, 128], F16, tag="h", bufs=NP + 2)
                        nc.scalar.copy(h, psh)
                        P[p]["h"] = h
                    for p in group:
                        psp = ps_sm.tile([128, 128], F32, tag="sm")
                        nc.tensor.matmul(psp, P[p]["wt"], P[p]["h"],
                                         start=True, stop=True)
                        u1 = smf.tile([128, 128], F32, tag="u1")
                        nc.vector.tensor_mul(u1, cmask, psp)
                        ctI = s16.tile([128, 128], F16, tag="ct",
                                       bufs=NP + 2)
                        nc.vector.tensor_sub(ctI, c15, u1)
                        P[p]["ctI"] = ctI
                    for p in group:
                        pswt = ps_sm.tile([128, 128], F32, tag="sm")
                        nc.tensor.matmul(pswt, P[p]["w"], P[p]["ctI"],
                                         start=True, stop=True)
                        if last:
                            wtn = wp.tile([128, 128], BF16, tag="wtb")
                            nc.scalar.copy(wtn, pswt)
                            P[p]["wt"] = wtn
                        else:
                            wtn = wp.tile([128, 128], F16, tag="wt")
                            nc.scalar.copy(wtn, pswt)
                            P[p]["wt"] = wtn
                            psw = ps_sm.tile([128, 128], F32, tag="sm")
                            nc.tensor.matmul(psw, P[p]["ctI"], P[p]["w"],
                                             start=True, stop=True)
                            wn = wp.tile([128, 128], F16, tag="w")
                            nc.scalar.copy(wn, psw)
                            P[p]["w"] = wn
                for p in group:
                    psv = ps_row.tile([128, 2], F32, tag="psv")
                    nc.tensor.matmul(psv[:, 0:1], P[p]["wt"], P[p]["mur"],
                                     start=True, stop=True)
                    biask = pp.tile([128, 1], F32, tag="biask")
                    nc.scalar.activation(out=biask, in_=psv[:, 0:1],
                                         func=AF.Identity, scale=-1.0)
                    P[p]["biask"] = biask

            # ---- Phase C (solve) ----
            def emit_solve(p):
                slab = slabp.tile([128, B], BF16, tag="slab")
                nc.sync.dma_start(out=slab, in_=d_xs.ap()[p])
                zpair = zp.tile([128, B], BF16, tag="zpair")
                wt, biask = P[p]["wt"], P[p]["biask"]
                for j in range(NB):
                    if j % 2 == 0:
                        psz = ps_big.tile([128, 512], F32, tag="big")
                    else:
                        psz = ps_sm.tile([128, 512], F32, tag="sm")
                    nc.tensor.matmul(psz, wt,
                                     slab[:, 512 * j:512 * (j + 1)],
                                     start=True, stop=True)
                    dst = zpair[:, 512 * j:512 * (j + 1)]
                    if j % 2 == 0:
                        nc.scalar.activation(out=dst, in_=psz,
                                             func=AF.Identity, bias=biask)
                    else:
                        nc.vector.tensor_scalar_add(dst, psz, biask)
                    if j == NB // 2 - 1:
                        nc.sync.dma_start(out=d_out.ap()[p][:, 0:B // 2],
                                          in_=zpair[:, 0:B // 2])
                nc.sync.dma_start(out=d_out.ap()[p][:, B // 2:B],
                                  in_=zpair[:, B // 2:B])

            # stats for all pairs first (DMA-paced), then newton+solve in
            # groups of 4 so out-DMA starts right after stats and flows
            GRP = 4
            for p in range(NP):
                emit_stats(p)
                if p >= 1:
                    emit_prep(p - 1)
            emit_prep(NP - 1)
            for g0 in range(0, NP, GRP):
                emit_newton(range(g0, g0 + GRP))
                for p in range(g0, g0 + GRP):
                    emit_solve(p)

    nc.finalize()
    return nc


def _make_consts():
    i64 = np.eye(64, dtype=np.float32)
    cm64 = np.triu(np.ones((64, 64), np.float32), 1) + 0.5 * i64
    z = np.zeros((64, 64), np.float32)
    cmask = np.block([[cm64, z], [z, cm64]])
    o64 = np.ones((64, 64), np.float32)
    blkm = np.block([[o64, z], [z, o64]])
    i128 = np.eye(128, dtype=np.float32)
    epsi = EPS * i128
    c15 = 1.5 * i128
    c15e = (1.5 - 0.5 * EPS) * i128
    return np.concatenate([cmask, blkm, epsi, c15, c15e], axis=1)


def _prep_inputs(x):
    """x: [B, D, K] fp32 -> per-core input dicts."""
    x = np.asarray(x, dtype=np.float32)
    consts = _make_consts()
    id16 = np.eye(128, dtype=np.float16)
    # xs: [K//2 pairs, 128, B]: rows c*64+d = x[:, d, 2p+c]
    xs_full = np.ascontiguousarray(
        x.transpose(2, 1, 0).reshape(K // 2, 128, B).astype(
            ml_dtypes.bfloat16))

    # xb (fp8 e4m3): [K//2, 128, NCH*130]: [i, 130j + c*64+d] =
    # x[128j+i, d, 2p+c], col 130j+128 = 1, col 130j+129 = 0
    a = x.reshape(NCH, 128, D, K // 2, 2)          # j, i, d, p, c
    a = a.transpose(3, 1, 0, 4, 2)                 # p, i, j, c, d
    xb_full = np.zeros((K // 2, 128, NCH, 130), dtype=ml_dtypes.float8_e4m3)
    xb_full[:, :, :, 0:128] = a.reshape(K // 2, 128, NCH, 128).astype(
        ml_dtypes.float8_e4m3)
    xb_full[:, :, :, 128] = np.float32(1.0)
    xb_full = xb_full.reshape(K // 2, 128, NCH * 130)

    in_maps = []
    for c in range(NCORES):
        ps = slice(c * NP, (c + 1) * NP)
        in_maps.append({"xb": np.ascontiguousarray(xb_full[ps]),
                        "xs": np.ascontiguousarray(xs_full[ps]),
                        "cs": consts, "id16": id16})
    return in_maps


def _run(x, trace=False):
    if "nc" not in _cache:
        _cache["nc"] = _build_nc()
    nc = _cache["nc"]
    in_maps = _prep_inputs(x)
    res = run_bass_kernel_spmd(nc, in_maps, core_ids=list(range(NCORES)),
                               trace=trace)
    out = np.empty((B, D, K), dtype=np.float32)
    for c in range(NCORES):
        ks = slice(c * KL, (c + 1) * KL)
        zo = np.asarray(res.results[c]["out"], dtype=np.float32)
        out[:, :, ks] = zo.reshape(NP, 2, 64, B).transpose(3, 2, 0, 1) \
                          .reshape(B, D, KL)
    return out, res


def kernel(x):
    out, _ = _run(x, trace=False)
    return out
